# revision 1
# baseline (speedup 1.0000x reference)
"""Trainium2 Bass kernel for nn_Net4 (hypernetwork RNN scan).

Model (per step t, per batch row b):
  h1 = sigmoid(m @ A1 + pre1[t])          A1 = W_enc_w[:64]
  h2 = sigmoid(m @ B1 + pre2[t])          B1 = b_enc_w[:64]
  Wm = (h1 @ W_dec_w + W_dec_b).reshape(64,64)
  bm = h2 @ b_dec_w + b_dec_b
  m' = sigmoid(Wm @ m + bm)
  loss[t] = (logsumexp(m'@dec_w+dec_b) - (m'@dec_w+dec_b)[y]) / ln2

pre1/pre2 are the window-dependent parts, precomputed on device via a
shifted-embedding matmul.  The bilinear Wm@m is reassociated as
  a[b,i] = sum_h h1[b,h] * T[b,h,i],  T[b,h,i] = sum_j W2r[h,i,j] m[b,j]
T is produced by 32 weight-stationary matmuls (chunk c covers i=c and
i=c+32 in the two partition halves), then contracted with h1 by 4 tiny
matmuls reading strided slices of T from SBUF.

Sharding: batch rows 2k,2k+1 -> core k; zero cross-core communication.
"""

import os
import sys
import numpy as np

sys.path.insert(0, "/opt/trn_rl_repo")

import concourse.bass as bass
import concourse.bacc as bacc
import concourse.mybir as mybir
import concourse.tile as tile
from concourse.bass_utils import run_bass_kernel_spmd

import ml_dtypes

BF16 = ml_dtypes.bfloat16

Cin, E, L, M, H, Cout = 256, 16, 64, 64, 64, 256
B, N = 16, 2048
D = M + L * E  # 1088
NCORES = 8
BL = B // NCORES  # 2 batch rows per core
NB = N * BL       # 4096 (t,b) pairs per core
TAU = N + L - 8   # e8 time length: tau in [0, 2104)
E8COLS = TAU * BL  # 4208

F32 = mybir.dt.float32
BF16_DT = mybir.dt.bfloat16
AF = mybir.ActivationFunctionType

_cache = {}


def _build_nc(unroll=16, staggered=False):
    nc = bacc.Bacc("TRN2", target_bir_lowering=False, debug=True)

    # ---- DRAM parameters (per-core inputs) ----
    def P(name, shape, dt):
        return nc.declare_dram_parameter(name, list(shape), dt, isOutput=False)

    e8_d = P("e8", (128, E8COLS), BF16_DT)
    wpre1_d = P("wpre1", (128, 8 * 128), BF16_DT)
    wpre2_d = P("wpre2", (128, 8 * 64), BF16_DT)
    bias1_d = P("bias1", (1, 128), BF16_DT)   # [W_enc_b | W_enc_b]
    bias2_d = P("bias2", (1, 64), BF16_DT)    # b_enc_b
    a1b1_d = P("a1b1", (128, 192), BF16_DT)
    wstatT_d = P("wstatT", (128, 16 * 128), BF16_DT)
    wbT_d = P("wbT", (64, 64), F32)           # W_dec_b reshaped [j,i]
    bw65_d = P("bw65", (65, 64), F32)         # [b_dec_w ; b_dec_b]
    decstat_d = P("decstat", (65, 256), F32)  # [dec_w ; dec_b]
    gaug_d = P("gaug", (65, NB), F32)
    ones65_d = P("ones65", (65, 1), F32)
    ones128_d = P("ones128", (128, 1), F32)
    out_d = nc.declare_dram_parameter("out", [1, NB], F32, isOutput=True)

    with tile.TileContext(nc) as tc:
        with (
            tc.tile_pool(name="persist", bufs=1) as pp,
            tc.tile_pool(name="psum", bufs=2, space="PSUM") as psp,
        ):
            e8 = pp.tile([128, E8COLS], BF16_DT)
            wpre1 = pp.tile([128, 8 * 128], BF16_DT)
            wpre2 = pp.tile([128, 8 * 64], BF16_DT)
            bias1 = pp.tile([1, 128], BF16_DT)
            bias2 = pp.tile([1, 64], BF16_DT)
            a1b1 = pp.tile([128, 192], BF16_DT)
            wstatT = pp.tile([128, 16 * 128], BF16_DT)
            wbT = pp.tile([64, 64], F32)
            bw65 = pp.tile([65, 64], F32)
            decstat = pp.tile([65, 256], F32)
            gaug = pp.tile([65, NB], F32)
            ones65 = pp.tile([65, 1], F32)
            ones128 = pp.tile([128, 1], F32)

            for sb, dr in [
                (e8, e8_d), (wpre1, wpre1_d), (wpre2, wpre2_d),
                (bias1, bias1_d), (bias2, bias2_d), (a1b1, a1b1_d),
                (wstatT, wstatT_d), (wbT, wbT_d),
                (bw65, bw65_d), (decstat, decstat_d),
                (gaug, gaug_d), (ones65, ones65_d), (ones128, ones128_d),
            ]:
                nc.default_dma_engine.dma_start(sb[:], dr[:])

            pre1 = pp.tile([128, NB], F32)
            pre2 = pp.tile([64, NB], F32)
            m_hist = pp.tile([65, NB + 2 * BL], F32)  # row 64 == 1.0
            m_bf = pp.tile([128, BL], BF16_DT)
            h2t = pp.tile([65, BL], F32)              # row 64 == 1.0
            onerow = pp.tile([1, 512], BF16_DT)

            nc.vector.memset(m_hist[0:64, 0:BL], 0.0)
            nc.vector.memset(m_hist[64:65, :], 1.0)
            nc.vector.memset(m_bf[:], 0.0)  # both halves
            nc.vector.memset(h2t[64:65, :], 1.0)
            nc.vector.memset(onerow[:], 1.0)

            # ---- precompute pre1/pre2 ----
            for n in range(8):
                ps1 = psp.tile([128, 512], F32, tag="pps")
                for c in range(8):
                    nc.tensor.matmul(
                        ps1[:],
                        wpre1[:, c * 128:(c + 1) * 128],
                        e8[:, 16 * c + 512 * n: 16 * c + 512 * n + 512],
                        start=(c == 0), stop=False,
                    )
                nc.tensor.matmul(ps1[:], bias1[:], onerow[:],
                                 start=False, stop=True, skip_group_check=True)
                nc.vector.tensor_copy(pre1[:, 512 * n: 512 * (n + 1)], ps1[:])
                ps2 = psp.tile([64, 512], F32, tag="pps")
                for c in range(8):
                    nc.tensor.matmul(
                        ps2[:],
                        wpre2[:, c * 64:(c + 1) * 64],
                        e8[:, 16 * c + 512 * n: 16 * c + 512 * n + 512],
                        start=(c == 0), stop=False,
                    )
                nc.tensor.matmul(ps2[:], bias2[:], onerow[:],
                                 start=False, stop=True, skip_group_check=True)
                nc.vector.tensor_copy(pre2[:, 512 * n: 512 * (n + 1)], ps2[:])

            # ---- the scan ----
            with (
                tc.tile_pool(name="scan_sb", bufs=2) as wp,
                tc.For_i(0, N, unroll, staggered_reset=staggered,
                         hint_engines=(mybir.EngineType.PE,)) as iv,
            ):
                for k in range(unroll):
                    tcol = (iv + k) * BL
                    g_ps = psp.tile([128, 4], F32, tag="g_ps")
                    T_pse = psp.tile([128, 16, BL], F32, tag="T_pse", bufs=1)
                    T_pso = psp.tile([128, 16, BL], F32, tag="T_pso", bufs=1)
                    a_ps = psp.tile([64, BL], F32, tag="a_ps")
                    h1p = wp.tile([128, BL], F32, tag="h1p")
                    h2p = wp.tile([64, BL], F32, tag="h2p")
                    h1d = wp.tile([128, BL], F32, tag="h1d")
                    tsb = wp.tile([128, 32, BL], F32, tag="tsb")

                    # g row-paired: h1-preact (dup) on rows 0-63, h2 on 64-127
                    nc.tensor.matmul(g_ps[:, 0:2], a1b1[0:64, 0:128],
                                     m_bf[0:64, :], start=True, stop=True,
                                     tile_position=(0, 0))
                    nc.tensor.matmul(g_ps[0:64, 2:4], a1b1[64:128, 128:192],
                                     m_bf[64:128, :], start=True, stop=True,
                                     tile_position=(64, 0))
                    # T chunks, interleaved across row groups
                    for p2 in range(16):
                        nc.tensor.matmul(
                            T_pse[:, p2, :],
                            wstatT[0:64, p2 * 128:(p2 + 1) * 128],
                            m_bf[0:64, :], start=True, stop=True,
                            tile_position=(0, 0))
                        nc.tensor.matmul(
                            T_pso[:, p2, :],
                            wstatT[64:128, p2 * 128:(p2 + 1) * 128],
                            m_bf[64:128, :], start=True, stop=True,
                            tile_position=(64, 0))
                    # h = sigmoid(g + pre)
                    nc.vector.tensor_tensor(h1p[:], g_ps[:, 0:2],
                                            pre1[:, bass.ds(tcol, BL)],
                                            mybir.AluOpType.add)
                    nc.scalar.activation(h1d[:], h1p[:], AF.Sigmoid)
                    nc.vector.tensor_tensor(h2p[:], g_ps[0:64, 2:4],
                                            pre2[:, bass.ds(tcol, BL)],
                                            mybir.AluOpType.add)
                    nc.scalar.activation(h2t[0:64, :], h2p[:], AF.Sigmoid)
                    # T -> SBUF: tsb[:, c, :] with c = 2*p2 + odd
                    nc.vector.tensor_copy(tsb[:, 0:32:2, :], T_pse[:])
                    nc.vector.tensor_copy(tsb[:, 1:32:2, :], T_pso[:])
                    # a = WbT@m + bw65@h2 + sum_h h1*T
                    nc.tensor.matmul(a_ps[:], wbT[:],
                                     m_hist[0:64, bass.ds(tcol, BL)],
                                     start=True, stop=False)
                    nc.tensor.matmul(a_ps[:], bw65[:], h2t[:],
                                     start=False, stop=False, skip_group_check=True)
                    for b in range(BL):
                        nc.tensor.matmul(a_ps[0:32, b: b + 1],
                                         tsb[0:64, :, b], h1d[0:64, b: b + 1],
                                         start=False, stop=False,
                                         skip_group_check=True,
                                         tile_position=(0, 0))
                        last = b == BL - 1
                        nc.tensor.matmul(a_ps[32:64, b: b + 1],
                                         tsb[64:128, :, b], h1d[64:128, b: b + 1],
                                         start=False, stop=last,
                                         skip_group_check=True,
                                         tile_position=(64, 32))
                    # m' = sigmoid(a): bf16 dup halves first (critical), then f32 hist
                    nc.scalar.activation(m_bf[0:64, :], a_ps[:], AF.Sigmoid)
                    nc.scalar.activation(m_bf[64:128, :], a_ps[:], AF.Sigmoid)
                    nc.scalar.activation(m_hist[0:64, bass.ds(tcol + BL, BL)],
                                         a_ps[:], AF.Sigmoid)

            # ---- bulk loss ----
            lse = pp.tile([1, NB], F32)
            paug = pp.tile([65, NB], F32)
            loss = pp.tile([1, NB], F32)
            nc.vector.tensor_copy(paug[64:65, :], gaug[64:65, :])
            nc.vector.tensor_tensor(paug[0:64, :], gaug[0:64, :],
                                    m_hist[0:64, BL: NB + BL],
                                    mybir.AluOpType.mult)
            with tc.tile_pool(name="bulk", bufs=2) as bp:
                for tcn in range(8):
                    sl = slice(512 * tcn, 512 * (tcn + 1))
                    se_ps = psp.tile([1, 512], F32, tag="pps")
                    for half in range(2):
                        lg_ps = psp.tile([128, 512], F32, tag="pps")
                        exps = bp.tile([128, 512], F32, tag="exps")
                        nc.tensor.matmul(
                            lg_ps[:],
                            decstat[:, half * 128:(half + 1) * 128],
                            m_hist[:, BL + 512 * tcn: BL + 512 * (tcn + 1)],
                            start=True, stop=True)
                        nc.scalar.activation(exps[:], lg_ps[:], AF.Exp)
                        nc.tensor.matmul(se_ps[:], ones128[:], exps[:],
                                         start=(half == 0), stop=(half == 1))
                    nc.scalar.activation(lse[:, sl], se_ps[:], AF.Ln)
                    pk_ps = psp.tile([1, 512], F32, tag="pps")
                    nc.tensor.matmul(pk_ps[:], ones65[:], paug[:, sl],
                                     start=True, stop=True)
                    nc.vector.tensor_tensor(loss[:, sl], lse[:, sl], pk_ps[:],
                                            mybir.AluOpType.subtract)
            nc.vector.tensor_scalar_mul(loss[:], loss[:],
                                        float(1.0 / np.log(2.0)))
            nc.default_dma_engine.dma_start(out_d[:], loss[:])

    nc.compile()
    return nc


def _prep_core_inputs(x0, emb, W_enc_w, W_enc_b, W_dec_w, W_dec_b,
                      b_enc_w, b_enc_b, b_dec_w, b_dec_b, dec_w, dec_b):
    """Host-side gathers/packing -> list of per-core input dicts."""
    f32 = np.float32
    x0 = np.asarray(x0)
    xp = np.concatenate([np.zeros((B, L), x0.dtype), x0], axis=1)  # [B, N+L]
    e = np.asarray(emb, f32)[xp]  # [B, N+L, E]

    # shared weight packs
    Wcat = np.concatenate([np.asarray(W_enc_w, f32), np.asarray(b_enc_w, f32)],
                          axis=1)  # [1088, 128]
    wpre1 = np.zeros((128, 8 * 128), f32)
    wpre2 = np.zeros((128, 8 * 64), f32)
    for c in range(8):
        blk = Wcat[64 + 128 * c: 64 + 128 * (c + 1)]  # [128, 128]
        wpre1[:, c * 128: c * 128 + 64] = blk[:, :64]
        wpre1[:, c * 128 + 64: c * 128 + 128] = blk[:, :64]
        wpre2[:, c * 64:(c + 1) * 64] = blk[:, 64:]
    bias1 = np.concatenate([np.asarray(W_enc_b, f32)] * 2).reshape(1, 128)
    bias2 = np.asarray(b_enc_b, f32).reshape(1, 64)
    a1b1 = np.zeros((128, 192), f32)
    a1b1[0:64, 0:128] = np.concatenate([Wcat[:64, :64]] * 2, axis=1)
    a1b1[64:128, 128:192] = Wcat[:64, 64:]

    W2r = np.asarray(W_dec_w, f32).reshape(H, M, M)  # [h, i, j]
    wstatT = np.zeros((128, 16 * 128), f32)
    for p2 in range(16):
        for half, c in ((0, 2 * p2), (1, 2 * p2 + 1)):
            rows = slice(64 * half, 64 * half + 64)
            wstatT[rows, p2 * 128: p2 * 128 + 64] = W2r[:, c, :].T
            wstatT[rows, p2 * 128 + 64: p2 * 128 + 128] = W2r[:, c + 32, :].T
    wbT = np.asarray(W_dec_b, f32).reshape(M, M).T.copy()  # [j, i]
    bw65 = np.concatenate([np.asarray(b_dec_w, f32),
                           np.asarray(b_dec_b, f32).reshape(1, 64)], axis=0)
    decstat = np.concatenate([np.asarray(dec_w, f32),
                              np.asarray(dec_b, f32).reshape(1, 256)], axis=0)
    ones65 = np.ones((65, 1), f32)
    ones128 = np.ones((128, 1), f32)

    shared = dict(
        wpre1=wpre1.astype(BF16), wpre2=wpre2.astype(BF16),
        bias1=bias1.astype(BF16), bias2=bias2.astype(BF16),
        a1b1=a1b1.astype(BF16),
        wstatT=wstatT.astype(BF16), wbT=wbT, bw65=bw65,
        decstat=decstat, ones65=ones65, ones128=ones128,
    )

    in_maps = []
    dec_wT = np.asarray(dec_w, f32).T.copy()  # [256, 64]
    dec_bv = np.asarray(dec_b, f32)
    for k in range(NCORES):
        rows = slice(BL * k, BL * (k + 1))
        ek = e[rows]  # [BL, N+L, E]
        # e8[l_sub*16+eps, tau*BL+b] = ek[b, tau+l_sub, eps]
        e8 = np.zeros((128, E8COLS), f32)
        for ls in range(8):
            # [BL, TAU, E] -> [E? ] want [eps, tau, b]
            blk = ek[:, ls: ls + TAU, :].transpose(2, 1, 0)  # [E, TAU, BL]
            e8[ls * 16:(ls + 1) * 16] = blk.reshape(E, E8COLS)
        y = np.asarray(x0[rows])  # [BL, N]
        g = dec_wT[y]  # [BL, N, 64]
        gaug = np.zeros((65, NB), f32)
        gaug[:64] = g.transpose(2, 1, 0).reshape(64, NB)
        gaug[64] = dec_bv[y].T.reshape(NB)
        d = dict(shared)
        d["e8"] = e8.astype(BF16)
        d["gaug"] = gaug
        in_maps.append(d)
    return in_maps


def kernel(**inputs):
    key = "nc"
    if key not in _cache:
        _cache[key] = _build_nc()
    nc = _cache[key]
    in_maps = _prep_core_inputs(**inputs)
    res = run_bass_kernel_spmd(nc, in_maps, list(range(NCORES)),
                               trace=bool(os.environ.get("KERNEL_TRACE")))
    _cache["last_result"] = res
    out = np.zeros((N, B), np.float32)
    for k in range(NCORES):
        out[:, BL * k: BL * (k + 1)] = res.results[k]["out"].reshape(N, BL)
    return out.reshape(-1)



# revision 3
# speedup vs baseline: 1.2248x; 1.2248x over previous
"""Trainium2 Bass kernel for nn_Net4 (hypernetwork RNN scan), v2.

Model (per step t, per batch row b):
  h1 = sigmoid(m @ A1 + pre1[t])          A1 = W_enc_w[:64]
  h2 = sigmoid(m @ B1 + pre2[t])          B1 = b_enc_w[:64]
  Wm = (h1 @ W_dec_w + W_dec_b).reshape(64,64)
  bm = h2 @ b_dec_w + b_dec_b
  m' = sigmoid(Wm @ m + bm)
  loss[t] = (logsumexp(m'@dec_w+dec_b) - (m'@dec_w+dec_b)[y]) / ln2

v2 changes vs v1:
  - everything in the scan is bf16 (v1 had fp32 wbT/bw65/tsb matmuls that
    cost 2x LDWEIGHTS+MATMUL at ~200ns each)
  - pre1/pre2 are pre-written into the g PSUM bank by the vector engine
    (interleaved preg layout), and the g matmuls accumulate on top with
    start=False; this removes the vector add from the critical path
  - single m_bf [64,2]: all T-chunk stationaries live on partitions 0-63
    so no duplicated m halves and only one sigmoid per step boundary
  - m_hist is a bf16 copy of m_bf done by the vector engine (off the
    critical path), feeding a bf16 bulk loss phase (v1 bulk was fp32 and
    took ~106us; bf16 takes ~15us)

Sharding: batch rows 2k,2k+1 -> core k; zero cross-core communication.
"""

import os
import sys
import numpy as np

sys.path.insert(0, "/opt/trn_rl_repo")

import concourse.bass as bass
import concourse.bacc as bacc
import concourse.mybir as mybir
import concourse.tile as tile
from concourse.bass_utils import run_bass_kernel_spmd

import ml_dtypes

BF16 = ml_dtypes.bfloat16

Cin, E, L, M, H, Cout = 256, 16, 64, 64, 64, 256
B, N = 16, 2048
D = M + L * E  # 1088
NCORES = 8
BL = B // NCORES  # 2 batch rows per core
NB = N * BL       # 4096 (t,b) pairs per core
TAU = N + L - 8   # e8 time length: tau in [0, 2104)
E8COLS = TAU * BL  # 4208

F32 = mybir.dt.float32
BF16_DT = mybir.dt.bfloat16
AF = mybir.ActivationFunctionType
INV_LN2 = float(1.0 / np.log(2.0))

_cache = {}


def _build_nc(unroll=16):
    nc = bacc.Bacc("TRN2", target_bir_lowering=False, debug=True)

    # ---- DRAM parameters (per-core inputs) ----
    def P(name, shape, dt):
        return nc.declare_dram_parameter(name, list(shape), dt, isOutput=False)

    e8_d = P("e8", (128, E8COLS), BF16_DT)
    wpre1_d = P("wpre1", (128, 8 * 128), BF16_DT)
    wpre2_d = P("wpre2", (128, 8 * 64), BF16_DT)
    bias1_d = P("bias1", (1, 128), BF16_DT)   # [W_enc_b | W_enc_b]
    bias2_d = P("bias2", (1, 64), BF16_DT)    # b_enc_b
    gse_d = P("gse", (64, 128), BF16_DT)      # [A1 | A1]
    gso_d = P("gso", (64, 128), BF16_DT)      # [B1 | B1]
    wstatT_d = P("wstatT", (64, 32 * 128), BF16_DT)
    wbT_d = P("wbT", (64, 64), BF16_DT)       # W_dec_b reshaped [j,i]
    bw65_d = P("bw65", (65, 64), BF16_DT)     # [b_dec_w ; b_dec_b]
    decstat_d = P("decstat", (65, 256), BF16_DT)  # [dec_w ; dec_b]
    gaug_d = P("gaug", (65, NB), BF16_DT)     # picked dec cols * inv_ln2
    ones65_d = P("ones65", (65, 1), BF16_DT)
    ones128_d = P("ones128", (128, 1), BF16_DT)
    out_d = nc.declare_dram_parameter("out", [1, NB], F32, isOutput=True)

    with tile.TileContext(nc) as tc:
        with (
            tc.tile_pool(name="persist", bufs=1) as pp,
            tc.tile_pool(name="psum", bufs=2, space="PSUM") as psp,
        ):
            e8 = pp.tile([128, E8COLS], BF16_DT)
            wpre1 = pp.tile([128, 8 * 128], BF16_DT)
            wpre2 = pp.tile([128, 8 * 64], BF16_DT)
            bias1 = pp.tile([1, 128], BF16_DT)
            bias2 = pp.tile([1, 64], BF16_DT)
            gse = pp.tile([64, 128], BF16_DT)
            gso = pp.tile([64, 128], BF16_DT)
            wstatT = pp.tile([64, 32 * 128], BF16_DT)
            wbT = pp.tile([64, 64], BF16_DT)
            bw65 = pp.tile([65, 64], BF16_DT)
            decstat = pp.tile([65, 256], BF16_DT)
            gaug = pp.tile([65, NB], BF16_DT)
            ones65 = pp.tile([65, 1], BF16_DT)
            ones128 = pp.tile([128, 1], BF16_DT)

            for sb, dr in [
                (e8, e8_d), (wpre1, wpre1_d), (wpre2, wpre2_d),
                (bias1, bias1_d), (bias2, bias2_d),
                (gse, gse_d), (gso, gso_d),
                (wstatT, wstatT_d), (wbT, wbT_d),
                (bw65, bw65_d), (decstat, decstat_d),
                (gaug, gaug_d), (ones65, ones65_d), (ones128, ones128_d),
            ]:
                nc.default_dma_engine.dma_start(sb[:], dr[:])

            preg = pp.tile([128, N, 4], F32)      # pre1 (dup) | pre2 per t
            m_hist = pp.tile([65, NB + 2 * BL], BF16_DT)  # row 64 == 1.0
            m_bf = pp.tile([64, BL], BF16_DT)
            h1d = pp.tile([128, BL], BF16_DT)
            h2t = pp.tile([65, BL], BF16_DT)      # row 64 == 1.0
            tsb = pp.tile([128, 32, BL], BF16_DT)
            onerow = pp.tile([1, 512], BF16_DT)
            zeros4 = pp.tile([64, 4], BF16_DT)

            nc.vector.memset(m_hist[0:64, 0:BL], 0.0)
            nc.vector.memset(m_hist[64:65, :], 1.0)
            nc.vector.memset(m_bf[:], 0.0)
            nc.vector.memset(h2t[64:65, :], 1.0)
            nc.vector.memset(onerow[:], 1.0)
            nc.vector.memset(zeros4[:], 0.0)

            # ---- precompute preg = [pre1(b0),pre1(b1),pre2(b0),pre2(b1)] ----
            for n in range(8):
                ps1 = psp.tile([128, 256, BL], F32, tag="big")
                for c in range(8):
                    nc.tensor.matmul(
                        ps1[:],
                        wpre1[:, c * 128:(c + 1) * 128],
                        e8[:, 16 * c + 512 * n: 16 * c + 512 * n + 512],
                        start=(c == 0), stop=False,
                    )
                nc.tensor.matmul(ps1[:], bias1[:], onerow[:],
                                 start=False, stop=True, skip_group_check=True)
                nc.vector.tensor_copy(preg[:, 256 * n:256 * (n + 1), 0:2],
                                      ps1[:])
                ps2 = psp.tile([64, 256, BL], F32, tag="big")
                for c in range(8):
                    nc.tensor.matmul(
                        ps2[:],
                        wpre2[:, c * 64:(c + 1) * 64],
                        e8[:, 16 * c + 512 * n: 16 * c + 512 * n + 512],
                        start=(c == 0), stop=False,
                    )
                nc.tensor.matmul(ps2[:], bias2[:], onerow[:],
                                 start=False, stop=True, skip_group_check=True)
                nc.vector.tensor_copy(preg[0:64, 256 * n:256 * (n + 1), 2:4],
                                      ps2[:])

            # warm up the two g PSUM banks so their has_written bits are set
            # (the in-loop g matmuls use start=False over a DVE pre-write)
            for _ in range(2):
                gw = psp.tile([128, 1, 4], F32, tag="g_ps")
                nc.tensor.matmul(gw[:, 0, :], gse[:], zeros4[:],
                                 start=True, stop=True)

            # ---- the scan ----
            with tc.For_i(0, N, unroll,
                          hint_engines=(mybir.EngineType.PE,)) as iv:
                for k in range(unroll):
                    t = iv + k
                    tcol = t * BL
                    g_ps = psp.tile([128, 1, 4], F32, tag="g_ps")
                    a_ps = psp.tile([64, BL], F32, tag="a_ps", bufs=1)
                    T_ps = psp.tile([128, 32, BL], F32, tag="T_ps", bufs=1)

                    # pre1/pre2 pre-written into the g bank (vector, early)
                    nc.vector.tensor_copy(g_ps[:], preg[:, bass.ds(t, 1), :])

                    # g = [A1|A1]^T m (+pre1), [B1|B1]^T m (+pre2)
                    nc.tensor.matmul(g_ps[:, 0, 0:2], gse[:], m_bf[:],
                                     start=False, stop=True,
                                     skip_group_check=True)
                    nc.tensor.matmul(g_ps[:, 0, 2:4], gso[:], m_bf[:],
                                     start=False, stop=True,
                                     skip_group_check=True)
                    # a = WbT@m + ...
                    nc.tensor.matmul(a_ps[:], wbT[:], m_bf[:],
                                     start=True, stop=False)
                    # T chunks: T_ps[p, c, b]: p<64 -> T[h=p, i=c],
                    #                          p>=64 -> T[h=p-64, i=c+32]
                    for c in range(32):
                        nc.tensor.matmul(
                            T_ps[:, c, :],
                            wstatT[:, c * 128:(c + 1) * 128],
                            m_bf[:], start=True, stop=True)

                    # h = sigmoid(g) (g already includes pre via pre-write)
                    nc.scalar.activation(h1d[:], g_ps[:, 0, 0:2], AF.Sigmoid)
                    nc.scalar.activation(h2t[0:64, :], g_ps[0:64, 0, 2:4],
                                         AF.Sigmoid)

                    # T -> SBUF (bf16)
                    nc.vector.tensor_copy(tsb[:, 0:16, :], T_ps[:, 0:16, :])
                    nc.vector.tensor_copy(tsb[:, 16:32, :], T_ps[:, 16:32, :])

                    # a += bw65@[h2;1] + sum_h h1*T
                    nc.tensor.matmul(a_ps[:], bw65[:], h2t[:],
                                     start=False, stop=False,
                                     skip_group_check=True)
                    for b in range(BL):
                        nc.tensor.matmul(a_ps[0:32, b: b + 1],
                                         tsb[0:64, :, b], h1d[0:64, b: b + 1],
                                         start=False, stop=False,
                                         skip_group_check=True,
                                         tile_position=(0, 0))
                        last = b == BL - 1
                        nc.tensor.matmul(a_ps[32:64, b: b + 1],
                                         tsb[64:128, :, b],
                                         h1d[64:128, b: b + 1],
                                         start=False, stop=last,
                                         skip_group_check=True,
                                         tile_position=(64, 32))

                    # m' = sigmoid(a)
                    nc.scalar.activation(m_bf[:], a_ps[:], AF.Sigmoid)
                    nc.vector.tensor_copy(
                        m_hist[0:64, bass.ds(tcol + BL, BL)], m_bf[:])

            # ---- bulk loss ----
            loss = pp.tile([1, NB], F32)
            with tc.tile_pool(name="bulk", bufs=2) as bp:
                for tcn in range(8):
                    sl = slice(512 * tcn, 512 * (tcn + 1))
                    msl = slice(BL + 512 * tcn, BL + 512 * (tcn + 1))
                    se_ps = psp.tile([1, 512], F32, tag="seps")
                    for half in range(2):
                        lg_ps = psp.tile([128, 512], F32, tag="big")
                        exps = bp.tile([128, 512], BF16_DT, tag="exps")
                        nc.tensor.matmul(
                            lg_ps[:],
                            decstat[:, half * 128:(half + 1) * 128],
                            m_hist[:, msl],
                            start=True, stop=True)
                        nc.scalar.activation(exps[:], lg_ps[:], AF.Exp)
                        nc.tensor.matmul(se_ps[:], ones128[:], exps[:],
                                         start=(half == 0), stop=(half == 1))
                    paug_t = bp.tile([65, 512], BF16_DT, tag="paug")
                    nc.vector.tensor_tensor(paug_t[:], gaug[:, sl],
                                            m_hist[:, msl],
                                            mybir.AluOpType.mult)
                    pk_ps = psp.tile([1, 512], F32, tag="seps")
                    nc.tensor.matmul(pk_ps[:], ones65[:], paug_t[:],
                                     start=True, stop=True)
                    lse_t = bp.tile([1, 512], F32, tag="lse")
                    nc.scalar.activation(lse_t[:], se_ps[:], AF.Ln)
                    # loss = lse*inv_ln2 - pick (pick already scaled on host)
                    nc.vector.scalar_tensor_tensor(
                        loss[:, sl], lse_t[:], INV_LN2, pk_ps[:],
                        mybir.AluOpType.mult, mybir.AluOpType.subtract)
            nc.default_dma_engine.dma_start(out_d[:], loss[:])

    nc.compile()
    return nc


def _prep_core_inputs(x0, emb, W_enc_w, W_enc_b, W_dec_w, W_dec_b,
                      b_enc_w, b_enc_b, b_dec_w, b_dec_b, dec_w, dec_b):
    """Host-side gathers/packing -> list of per-core input dicts."""
    f32 = np.float32
    x0 = np.asarray(x0)
    xp = np.concatenate([np.zeros((B, L), x0.dtype), x0], axis=1)  # [B, N+L]
    e = np.asarray(emb, f32)[xp]  # [B, N+L, E]

    # shared weight packs
    Wcat = np.concatenate([np.asarray(W_enc_w, f32), np.asarray(b_enc_w, f32)],
                          axis=1)  # [1088, 128]
    wpre1 = np.zeros((128, 8 * 128), f32)
    wpre2 = np.zeros((128, 8 * 64), f32)
    for c in range(8):
        blk = Wcat[64 + 128 * c: 64 + 128 * (c + 1)]  # [128, 128]
        wpre1[:, c * 128: c * 128 + 64] = blk[:, :64]
        wpre1[:, c * 128 + 64: c * 128 + 128] = blk[:, :64]
        wpre2[:, c * 64:(c + 1) * 64] = blk[:, 64:]
    bias1 = np.concatenate([np.asarray(W_enc_b, f32)] * 2).reshape(1, 128)
    bias2 = np.asarray(b_enc_b, f32).reshape(1, 64)
    gse = np.concatenate([Wcat[:64, :64]] * 2, axis=1)    # [64, 128]
    gso = np.concatenate([Wcat[:64, 64:128]] * 2, axis=1)  # [64, 128]

    W2r = np.asarray(W_dec_w, f32).reshape(H, M, M)  # [h, i, j]
    wstatT = np.zeros((64, 32 * 128), f32)
    for c in range(32):
        wstatT[:, c * 128: c * 128 + 64] = W2r[:, c, :].T
        wstatT[:, c * 128 + 64: c * 128 + 128] = W2r[:, c + 32, :].T
    wbT = np.asarray(W_dec_b, f32).reshape(M, M).T.copy()  # [j, i]
    bw65 = np.concatenate([np.asarray(b_dec_w, f32),
                           np.asarray(b_dec_b, f32).reshape(1, 64)], axis=0)
    decstat = np.concatenate([np.asarray(dec_w, f32),
                              np.asarray(dec_b, f32).reshape(1, 256)], axis=0)
    ones65 = np.ones((65, 1), f32)
    ones128 = np.ones((128, 1), f32)

    shared = dict(
        wpre1=wpre1.astype(BF16), wpre2=wpre2.astype(BF16),
        bias1=bias1.astype(BF16), bias2=bias2.astype(BF16),
        gse=gse.astype(BF16), gso=gso.astype(BF16),
        wstatT=wstatT.astype(BF16), wbT=wbT.astype(BF16),
        bw65=bw65.astype(BF16),
        decstat=decstat.astype(BF16),
        ones65=ones65.astype(BF16), ones128=ones128.astype(BF16),
    )

    in_maps = []
    inv_ln2 = np.float32(1.0 / np.log(2.0))
    dec_wT = np.asarray(dec_w, f32).T.copy()  # [256, 64]
    dec_bv = np.asarray(dec_b, f32)
    for k in range(NCORES):
        rows = slice(BL * k, BL * (k + 1))
        ek = e[rows]  # [BL, N+L, E]
        # e8[l_sub*16+eps, tau*BL+b] = ek[b, tau+l_sub, eps]
        e8 = np.zeros((128, E8COLS), f32)
        for ls in range(8):
            blk = ek[:, ls: ls + TAU, :].transpose(2, 1, 0)  # [E, TAU, BL]
            e8[ls * 16:(ls + 1) * 16] = blk.reshape(E, E8COLS)
        y = np.asarray(x0[rows])  # [BL, N]
        g = dec_wT[y]  # [BL, N, 64]
        gaug = np.zeros((65, NB), f32)
        gaug[:64] = g.transpose(2, 1, 0).reshape(64, NB)
        gaug[64] = dec_bv[y].T.reshape(NB)
        gaug *= inv_ln2
        d = dict(shared)
        d["e8"] = e8.astype(BF16)
        d["gaug"] = gaug.astype(BF16)
        in_maps.append(d)
    return in_maps


def kernel(**inputs):
    key = "nc"
    if key not in _cache:
        _cache[key] = _build_nc()
    nc = _cache[key]
    in_maps = _prep_core_inputs(**inputs)
    res = run_bass_kernel_spmd(nc, in_maps, list(range(NCORES)),
                               trace=bool(os.environ.get("KERNEL_TRACE")))
    _cache["last_result"] = res
    out = np.zeros((N, B), np.float32)
    for k in range(NCORES):
        out[:, BL * k: BL * (k + 1)] = res.results[k]["out"].reshape(N, BL)
    return out.reshape(-1)


# revision 6
# speedup vs baseline: 1.4346x; 1.1712x over previous
"""Trainium2 Bass kernel for nn_Net4 (hypernetwork RNN scan), v2.

Model (per step t, per batch row b):
  h1 = sigmoid(m @ A1 + pre1[t])          A1 = W_enc_w[:64]
  h2 = sigmoid(m @ B1 + pre2[t])          B1 = b_enc_w[:64]
  Wm = (h1 @ W_dec_w + W_dec_b).reshape(64,64)
  bm = h2 @ b_dec_w + b_dec_b
  m' = sigmoid(Wm @ m + bm)
  loss[t] = (logsumexp(m'@dec_w+dec_b) - (m'@dec_w+dec_b)[y]) / ln2

v2 changes vs v1:
  - everything in the scan is bf16 (v1 had fp32 wbT/bw65/tsb matmuls that
    cost 2x LDWEIGHTS+MATMUL at ~200ns each)
  - pre1/pre2 are pre-written into the g PSUM bank by the vector engine
    (interleaved preg layout), and the g matmuls accumulate on top with
    start=False; this removes the vector add from the critical path
  - single m_bf [64,2]: all T-chunk stationaries live on partitions 0-63
    so no duplicated m halves and only one sigmoid per step boundary
  - m_hist is a bf16 copy of m_bf done by the vector engine (off the
    critical path), feeding a bf16 bulk loss phase (v1 bulk was fp32 and
    took ~106us; bf16 takes ~15us)

Sharding: batch rows 2k,2k+1 -> core k; zero cross-core communication.
"""

import os
import sys
import numpy as np

sys.path.insert(0, "/opt/trn_rl_repo")

import concourse.bass as bass
import concourse.bacc as bacc
import concourse.mybir as mybir
import concourse.tile as tile
from concourse.bass_utils import run_bass_kernel_spmd

import ml_dtypes

BF16 = ml_dtypes.bfloat16

Cin, E, L, M, H, Cout = 256, 16, 64, 64, 64, 256
B, N = 16, 2048
D = M + L * E  # 1088
NCORES = 8
BL = B // NCORES  # 2 batch rows per core
NB = N * BL       # 4096 (t,b) pairs per core
TAU = N + L - 8   # e8 time length: tau in [0, 2104)
E8COLS = TAU * BL  # 4208

F32 = mybir.dt.float32
BF16_DT = mybir.dt.bfloat16
AF = mybir.ActivationFunctionType
INV_LN2 = float(1.0 / np.log(2.0))

_cache = {}


def _build_nc(unroll=16):
    nc = bacc.Bacc("TRN2", target_bir_lowering=False, debug=True)

    # ---- DRAM parameters (per-core inputs) ----
    def P(name, shape, dt):
        return nc.declare_dram_parameter(name, list(shape), dt, isOutput=False)

    e8_d = P("e8", (128, E8COLS), BF16_DT)
    wpre1_d = P("wpre1", (128, 8 * 128), BF16_DT)
    wpre2_d = P("wpre2", (128, 8 * 64), BF16_DT)
    bias1_d = P("bias1", (1, 128), BF16_DT)   # [W_enc_b | W_enc_b]
    bias2_d = P("bias2", (1, 64), BF16_DT)    # b_enc_b
    gse_d = P("gse", (64, 128), BF16_DT)      # [A1 | A1]
    gso_d = P("gso", (64, 128), BF16_DT)      # [B1 | B1]
    wstatT_d = P("wstatT", (64, 32 * 128), BF16_DT)
    wbT_d = P("wbT", (64, 64), BF16_DT)       # W_dec_b reshaped [j,i]
    bw65_d = P("bw65", (65, 64), BF16_DT)     # [b_dec_w ; b_dec_b]
    decstat_d = P("decstat", (65, 256), BF16_DT)  # [dec_w ; dec_b]
    gaug_d = P("gaug", (65, NB), BF16_DT)     # picked dec cols * inv_ln2
    ones65_d = P("ones65", (65, 1), BF16_DT)
    ones128_d = P("ones128", (128, 1), BF16_DT)
    out_d = nc.declare_dram_parameter("out", [1, NB], F32, isOutput=True)

    with tile.TileContext(nc) as tc:
        with (
            tc.tile_pool(name="persist", bufs=1) as pp,
            tc.tile_pool(name="psum", bufs=2, space="PSUM") as psp,
        ):
            e8 = pp.tile([128, E8COLS], BF16_DT)
            wpre1 = pp.tile([128, 8 * 128], BF16_DT)
            wpre2 = pp.tile([128, 8 * 64], BF16_DT)
            bias1 = pp.tile([1, 128], BF16_DT)
            bias2 = pp.tile([1, 64], BF16_DT)
            gse = pp.tile([64, 128], BF16_DT)
            gso = pp.tile([64, 128], BF16_DT)
            wstatT = pp.tile([64, 32 * 128], BF16_DT)
            wbT = pp.tile([64, 64], BF16_DT)
            bw65 = pp.tile([65, 64], BF16_DT)
            decstat = pp.tile([65, 256], BF16_DT)
            gaug = pp.tile([65, NB], BF16_DT)
            ones65 = pp.tile([65, 1], BF16_DT)
            ones128 = pp.tile([128, 1], BF16_DT)

            for sb, dr in [
                (e8, e8_d), (wpre1, wpre1_d), (wpre2, wpre2_d),
                (bias1, bias1_d), (bias2, bias2_d),
                (gse, gse_d), (gso, gso_d),
                (wstatT, wstatT_d), (wbT, wbT_d),
                (bw65, bw65_d), (decstat, decstat_d),
                (gaug, gaug_d), (ones65, ones65_d), (ones128, ones128_d),
            ]:
                nc.default_dma_engine.dma_start(sb[:], dr[:])

            preg = pp.tile([128, N, 4], F32)      # pre1 (dup) | pre2 per t
            m_hist = pp.tile([65, NB + 2 * BL], BF16_DT)  # row 64 == 1.0
            m_bf = pp.tile([64, BL], BF16_DT)
            h1d = pp.tile([128, BL], BF16_DT)
            h2t = pp.tile([65, BL], BF16_DT)      # row 64 == 1.0
            tsb = pp.tile([128, 32, BL], BF16_DT)
            onerow = pp.tile([1, 512], BF16_DT)
            zeros4 = pp.tile([64, 4], BF16_DT)

            nc.vector.memset(m_hist[0:64, 0:BL], 0.0)
            nc.vector.memset(m_hist[64:65, :], 1.0)
            nc.vector.memset(m_bf[:], 0.0)
            nc.vector.memset(h2t[64:65, :], 1.0)
            nc.vector.memset(onerow[:], 1.0)
            nc.vector.memset(zeros4[:], 0.0)

            # ---- precompute preg = [pre1(b0),pre1(b1),pre2(b0),pre2(b1)] ----
            for n in range(8):
                ps1 = psp.tile([128, 256, BL], F32, tag="big")
                for c in range(8):
                    nc.tensor.matmul(
                        ps1[:],
                        wpre1[:, c * 128:(c + 1) * 128],
                        e8[:, 16 * c + 512 * n: 16 * c + 512 * n + 512],
                        start=(c == 0), stop=False,
                    )
                nc.tensor.matmul(ps1[:], bias1[:], onerow[:],
                                 start=False, stop=True, skip_group_check=True)
                nc.vector.tensor_copy(preg[:, 256 * n:256 * (n + 1), 0:2],
                                      ps1[:])
                ps2 = psp.tile([64, 256, BL], F32, tag="big")
                for c in range(8):
                    nc.tensor.matmul(
                        ps2[:],
                        wpre2[:, c * 64:(c + 1) * 64],
                        e8[:, 16 * c + 512 * n: 16 * c + 512 * n + 512],
                        start=(c == 0), stop=False,
                    )
                nc.tensor.matmul(ps2[:], bias2[:], onerow[:],
                                 start=False, stop=True, skip_group_check=True)
                nc.vector.tensor_copy(preg[0:64, 256 * n:256 * (n + 1), 2:4],
                                      ps2[:])

            # warm up the two g PSUM banks so their has_written bits are set
            # (the in-loop g matmuls use start=False over a DVE pre-write)
            for _ in range(2):
                gw = psp.tile([128, 1, 4], F32, tag="g_ps")
                nc.tensor.matmul(gw[:, 0, :], gse[:], zeros4[:],
                                 start=True, stop=True)

            # ---- the scan ----
            with tc.For_i(0, N, unroll,
                          hint_engines=(mybir.EngineType.PE,)) as iv:
                for k in range(unroll):
                    t = iv + k
                    tcol = t * BL
                    g_ps = psp.tile([128, 1, 4], F32, tag="g_ps")
                    a_ps = psp.tile([64, BL], F32, tag="a_ps", bufs=1)
                    T_psA = psp.tile([128, 16, BL], F32, tag="T_psA", bufs=1)
                    T_psB = psp.tile([128, 16, BL], F32, tag="T_psB", bufs=1)

                    # pre1/pre2 pre-written into the g bank (vector, early)
                    nc.vector.tensor_copy(g_ps[:], preg[:, bass.ds(t, 1), :])

                    # g = [A1|A1]^T m (+pre1), [B1|B1]^T m (+pre2)
                    nc.tensor.matmul(g_ps[:, 0, 0:2], gse[:], m_bf[:],
                                     start=False, stop=True,
                                     skip_group_check=True)
                    nc.tensor.matmul(g_ps[:, 0, 2:4], gso[:], m_bf[:],
                                     start=False, stop=True,
                                     skip_group_check=True)
                    # T chunks: T[p, c, b]: p<64 -> T[h=p, i=c],
                    #                       p>=64 -> T[h=p-64, i=c+32]
                    for c in range(16):
                        nc.tensor.matmul(
                            T_psA[:, c, :],
                            wstatT[:, c * 128:(c + 1) * 128],
                            m_bf[:], start=True, stop=True)
                    for c in range(16, 32):
                        nc.tensor.matmul(
                            T_psB[:, c - 16, :],
                            wstatT[:, c * 128:(c + 1) * 128],
                            m_bf[:], start=True, stop=True)

                    # h = sigmoid(g) (g already includes pre via pre-write)
                    nc.scalar.activation(h1d[:], g_ps[:, 0, 0:2], AF.Sigmoid)
                    nc.scalar.activation(h2t[0:64, :], g_ps[0:64, 0, 2:4],
                                         AF.Sigmoid)

                    # T -> SBUF (bf16); A copy overlaps the B-chunk matmuls
                    nc.vector.tensor_copy(tsb[:, 0:16, :], T_psA[:])
                    nc.vector.tensor_copy(tsb[:, 16:32, :], T_psB[:])

                    # a = WbT@m + bw65@[h2;1] + sum_h h1*T
                    nc.tensor.matmul(a_ps[:], wbT[:], m_bf[:],
                                     start=True, stop=False)
                    nc.tensor.matmul(a_ps[:], bw65[:], h2t[:],
                                     start=False, stop=False,
                                     skip_group_check=True)
                    for b in range(BL):
                        nc.tensor.matmul(a_ps[0:32, b: b + 1],
                                         tsb[0:64, :, b], h1d[0:64, b: b + 1],
                                         start=False, stop=False,
                                         skip_group_check=True,
                                         tile_position=(0, 0))
                        last = b == BL - 1
                        nc.tensor.matmul(a_ps[32:64, b: b + 1],
                                         tsb[64:128, :, b],
                                         h1d[64:128, b: b + 1],
                                         start=False, stop=last,
                                         skip_group_check=True,
                                         tile_position=(64, 32))

                    # m' = sigmoid(a)
                    nc.scalar.activation(m_bf[:], a_ps[:], AF.Sigmoid)
                    nc.vector.tensor_copy(
                        m_hist[0:64, bass.ds(tcol + BL, BL)], m_bf[:])

            # ---- bulk loss ----
            loss = pp.tile([1, NB], F32)
            with tc.tile_pool(name="bulk", bufs=2) as bp:
                for tcn in range(8):
                    sl = slice(512 * tcn, 512 * (tcn + 1))
                    msl = slice(BL + 512 * tcn, BL + 512 * (tcn + 1))
                    se_ps = psp.tile([1, 512], F32, tag="seps", bufs=1)
                    for half in range(2):
                        lg_ps = psp.tile([128, 512], F32, tag="big")
                        exps = bp.tile([128, 512], BF16_DT, tag="exps")
                        nc.tensor.matmul(
                            lg_ps[:],
                            decstat[:, half * 128:(half + 1) * 128],
                            m_hist[:, msl],
                            start=True, stop=True)
                        nc.scalar.activation(exps[:], lg_ps[:], AF.Exp)
                        nc.tensor.matmul(se_ps[:], ones128[:], exps[:],
                                         start=(half == 0), stop=(half == 1))
                    lse_t = bp.tile([1, 512], F32, tag="lse")
                    nc.scalar.activation(lse_t[:], se_ps[:], AF.Ln)
                    paug_t = bp.tile([65, 512], BF16_DT, tag="paug")
                    nc.vector.tensor_tensor(paug_t[:], gaug[:, sl],
                                            m_hist[:, msl],
                                            mybir.AluOpType.mult)
                    pk_ps = psp.tile([1, 512], F32, tag="seps", bufs=1)
                    nc.tensor.matmul(pk_ps[:], ones65[:], paug_t[:],
                                     start=True, stop=True)
                    # loss = lse*inv_ln2 - pick (pick already scaled on host)
                    nc.vector.scalar_tensor_tensor(
                        loss[:, sl], lse_t[:], INV_LN2, pk_ps[:],
                        mybir.AluOpType.mult, mybir.AluOpType.subtract)
            nc.default_dma_engine.dma_start(out_d[:], loss[:])

    nc.compile()
    return nc


def _prep_core_inputs(x0, emb, W_enc_w, W_enc_b, W_dec_w, W_dec_b,
                      b_enc_w, b_enc_b, b_dec_w, b_dec_b, dec_w, dec_b):
    """Host-side gathers/packing -> list of per-core input dicts."""
    f32 = np.float32
    x0 = np.asarray(x0)
    xp = np.concatenate([np.zeros((B, L), x0.dtype), x0], axis=1)  # [B, N+L]
    e = np.asarray(emb, f32)[xp]  # [B, N+L, E]

    # shared weight packs
    Wcat = np.concatenate([np.asarray(W_enc_w, f32), np.asarray(b_enc_w, f32)],
                          axis=1)  # [1088, 128]
    wpre1 = np.zeros((128, 8 * 128), f32)
    wpre2 = np.zeros((128, 8 * 64), f32)
    for c in range(8):
        blk = Wcat[64 + 128 * c: 64 + 128 * (c + 1)]  # [128, 128]
        wpre1[:, c * 128: c * 128 + 64] = blk[:, :64]
        wpre1[:, c * 128 + 64: c * 128 + 128] = blk[:, :64]
        wpre2[:, c * 64:(c + 1) * 64] = blk[:, 64:]
    bias1 = np.concatenate([np.asarray(W_enc_b, f32)] * 2).reshape(1, 128)
    bias2 = np.asarray(b_enc_b, f32).reshape(1, 64)
    gse = np.concatenate([Wcat[:64, :64]] * 2, axis=1)    # [64, 128]
    gso = np.concatenate([Wcat[:64, 64:128]] * 2, axis=1)  # [64, 128]

    W2r = np.asarray(W_dec_w, f32).reshape(H, M, M)  # [h, i, j]
    wstatT = np.zeros((64, 32 * 128), f32)
    for c in range(32):
        wstatT[:, c * 128: c * 128 + 64] = W2r[:, c, :].T
        wstatT[:, c * 128 + 64: c * 128 + 128] = W2r[:, c + 32, :].T
    wbT = np.asarray(W_dec_b, f32).reshape(M, M).T.copy()  # [j, i]
    bw65 = np.concatenate([np.asarray(b_dec_w, f32),
                           np.asarray(b_dec_b, f32).reshape(1, 64)], axis=0)
    decstat = np.concatenate([np.asarray(dec_w, f32),
                              np.asarray(dec_b, f32).reshape(1, 256)], axis=0)
    ones65 = np.ones((65, 1), f32)
    ones128 = np.ones((128, 1), f32)

    shared = dict(
        wpre1=wpre1.astype(BF16), wpre2=wpre2.astype(BF16),
        bias1=bias1.astype(BF16), bias2=bias2.astype(BF16),
        gse=gse.astype(BF16), gso=gso.astype(BF16),
        wstatT=wstatT.astype(BF16), wbT=wbT.astype(BF16),
        bw65=bw65.astype(BF16),
        decstat=decstat.astype(BF16),
        ones65=ones65.astype(BF16), ones128=ones128.astype(BF16),
    )

    in_maps = []
    inv_ln2 = np.float32(1.0 / np.log(2.0))
    dec_wT = np.asarray(dec_w, f32).T.copy()  # [256, 64]
    dec_bv = np.asarray(dec_b, f32)
    for k in range(NCORES):
        rows = slice(BL * k, BL * (k + 1))
        ek = e[rows]  # [BL, N+L, E]
        # e8[l_sub*16+eps, tau*BL+b] = ek[b, tau+l_sub, eps]
        e8 = np.zeros((128, E8COLS), f32)
        for ls in range(8):
            blk = ek[:, ls: ls + TAU, :].transpose(2, 1, 0)  # [E, TAU, BL]
            e8[ls * 16:(ls + 1) * 16] = blk.reshape(E, E8COLS)
        y = np.asarray(x0[rows])  # [BL, N]
        g = dec_wT[y]  # [BL, N, 64]
        gaug = np.zeros((65, NB), f32)
        gaug[:64] = g.transpose(2, 1, 0).reshape(64, NB)
        gaug[64] = dec_bv[y].T.reshape(NB)
        gaug *= inv_ln2
        d = dict(shared)
        d["e8"] = e8.astype(BF16)
        d["gaug"] = gaug.astype(BF16)
        in_maps.append(d)
    return in_maps


def kernel(**inputs):
    key = "nc"
    if key not in _cache:
        _cache[key] = _build_nc()
    nc = _cache[key]
    in_maps = _prep_core_inputs(**inputs)
    res = run_bass_kernel_spmd(nc, in_maps, list(range(NCORES)),
                               trace=bool(os.environ.get("KERNEL_TRACE")))
    _cache["last_result"] = res
    out = np.zeros((N, B), np.float32)
    for k in range(NCORES):
        out[:, BL * k: BL * (k + 1)] = res.results[k]["out"].reshape(N, BL)
    return out.reshape(-1)


# revision 7
# speedup vs baseline: 1.4349x; 1.0002x over previous
"""Trainium2 Bass kernel for nn_Net4 (hypernetwork RNN scan), v2.

Model (per step t, per batch row b):
  h1 = sigmoid(m @ A1 + pre1[t])          A1 = W_enc_w[:64]
  h2 = sigmoid(m @ B1 + pre2[t])          B1 = b_enc_w[:64]
  Wm = (h1 @ W_dec_w + W_dec_b).reshape(64,64)
  bm = h2 @ b_dec_w + b_dec_b
  m' = sigmoid(Wm @ m + bm)
  loss[t] = (logsumexp(m'@dec_w+dec_b) - (m'@dec_w+dec_b)[y]) / ln2

v2 changes vs v1:
  - everything in the scan is bf16 (v1 had fp32 wbT/bw65/tsb matmuls that
    cost 2x LDWEIGHTS+MATMUL at ~200ns each)
  - pre1/pre2 are pre-written into the g PSUM bank by the vector engine
    (interleaved preg layout), and the g matmuls accumulate on top with
    start=False; this removes the vector add from the critical path
  - single m_bf [64,2]: all T-chunk stationaries live on partitions 0-63
    so no duplicated m halves and only one sigmoid per step boundary
  - m_hist is a bf16 copy of m_bf done by the vector engine (off the
    critical path), feeding a bf16 bulk loss phase (v1 bulk was fp32 and
    took ~106us; bf16 takes ~15us)

Sharding: batch rows 2k,2k+1 -> core k; zero cross-core communication.
"""

import os
import sys
import numpy as np

sys.path.insert(0, "/opt/trn_rl_repo")

import concourse.bass as bass
import concourse.bacc as bacc
import concourse.mybir as mybir
import concourse.tile as tile
from concourse.bass_utils import run_bass_kernel_spmd

import ml_dtypes

BF16 = ml_dtypes.bfloat16

Cin, E, L, M, H, Cout = 256, 16, 64, 64, 64, 256
B, N = 16, 2048
D = M + L * E  # 1088
NCORES = 8
BL = B // NCORES  # 2 batch rows per core
NB = N * BL       # 4096 (t,b) pairs per core
TAU = N + L - 8   # e8 time length: tau in [0, 2104)
E8COLS = TAU * BL  # 4208

F32 = mybir.dt.float32
BF16_DT = mybir.dt.bfloat16
AF = mybir.ActivationFunctionType
FP8_DT = mybir.dt.float8e4
FP8 = ml_dtypes.float8_e4m3fn
INV_LN2 = float(1.0 / np.log(2.0))

_cache = {}


def _build_nc(unroll=16):
    nc = bacc.Bacc("TRN2", target_bir_lowering=False, debug=True)

    # ---- DRAM parameters (per-core inputs) ----
    def P(name, shape, dt):
        return nc.declare_dram_parameter(name, list(shape), dt, isOutput=False)

    e8_d = P("e8", (128, E8COLS), BF16_DT)
    wpre1_d = P("wpre1", (128, 8 * 128), BF16_DT)
    wpre2_d = P("wpre2", (128, 8 * 64), BF16_DT)
    bias1_d = P("bias1", (1, 128), BF16_DT)   # [W_enc_b | W_enc_b]
    bias2_d = P("bias2", (1, 64), BF16_DT)    # b_enc_b
    gse_d = P("gse", (64, 128), BF16_DT)      # [A1 | A1]
    gso_d = P("gso", (64, 128), BF16_DT)      # [B1 | B1]
    wstatT_d = P("wstatT", (64, 32 * 128), FP8_DT)
    wbT_d = P("wbT", (64, 64), BF16_DT)       # W_dec_b reshaped [j,i]
    bw65_d = P("bw65", (65, 64), BF16_DT)     # [b_dec_w ; b_dec_b]
    decstat_d = P("decstat", (65, 256), BF16_DT)  # [dec_w ; dec_b]
    gaug_d = P("gaug", (65, NB), BF16_DT)     # picked dec cols * inv_ln2
    ones65_d = P("ones65", (65, 1), BF16_DT)
    ones128_d = P("ones128", (128, 1), BF16_DT)
    out_d = nc.declare_dram_parameter("out", [1, NB], F32, isOutput=True)

    with tile.TileContext(nc) as tc:
        with (
            tc.tile_pool(name="persist", bufs=1) as pp,
            tc.tile_pool(name="psum", bufs=2, space="PSUM") as psp,
        ):
            e8 = pp.tile([128, E8COLS], BF16_DT)
            wpre1 = pp.tile([128, 8 * 128], BF16_DT)
            wpre2 = pp.tile([128, 8 * 64], BF16_DT)
            bias1 = pp.tile([1, 128], BF16_DT)
            bias2 = pp.tile([1, 64], BF16_DT)
            gse = pp.tile([64, 128], BF16_DT)
            gso = pp.tile([64, 128], BF16_DT)
            wstatT = pp.tile([64, 32 * 128], FP8_DT)
            wbT = pp.tile([64, 64], BF16_DT)
            bw65 = pp.tile([65, 64], BF16_DT)
            decstat = pp.tile([65, 256], BF16_DT)
            gaug = pp.tile([65, NB], BF16_DT)
            ones65 = pp.tile([65, 1], BF16_DT)
            ones128 = pp.tile([128, 1], BF16_DT)

            for sb, dr in [
                (e8, e8_d), (wpre1, wpre1_d), (wpre2, wpre2_d),
                (bias1, bias1_d), (bias2, bias2_d),
                (gse, gse_d), (gso, gso_d),
                (wstatT, wstatT_d), (wbT, wbT_d),
                (bw65, bw65_d), (decstat, decstat_d),
                (gaug, gaug_d), (ones65, ones65_d), (ones128, ones128_d),
            ]:
                nc.default_dma_engine.dma_start(sb[:], dr[:])

            preg = pp.tile([128, N, 4], F32)      # pre1 (dup) | pre2 per t
            m_hist = pp.tile([65, NB + 2 * BL], BF16_DT)  # row 64 == 1.0
            m_bf = pp.tile([64, BL], BF16_DT)
            h1d = pp.tile([128, BL], BF16_DT)
            h2t = pp.tile([65, BL], BF16_DT)      # row 64 == 1.0
            tsb = pp.tile([128, 32, BL], BF16_DT)
            onerow = pp.tile([1, 512], BF16_DT)
            zeros4 = pp.tile([64, 4], BF16_DT)

            nc.vector.memset(m_hist[0:64, 0:BL], 0.0)
            nc.vector.memset(m_hist[64:65, :], 1.0)
            nc.vector.memset(m_bf[:], 0.0)
            nc.vector.memset(h2t[64:65, :], 1.0)
            nc.vector.memset(onerow[:], 1.0)
            nc.vector.memset(zeros4[:], 0.0)

            # ---- precompute preg = [pre1(b0),pre1(b1),pre2(b0),pre2(b1)] ----
            for n in range(8):
                ps1 = psp.tile([128, 256, BL], F32, tag="big")
                for c in range(8):
                    nc.tensor.matmul(
                        ps1[:],
                        wpre1[:, c * 128:(c + 1) * 128],
                        e8[:, 16 * c + 512 * n: 16 * c + 512 * n + 512],
                        start=(c == 0), stop=False,
                    )
                nc.tensor.matmul(ps1[:], bias1[:], onerow[:],
                                 start=False, stop=True, skip_group_check=True)
                nc.vector.tensor_copy(preg[:, 256 * n:256 * (n + 1), 0:2],
                                      ps1[:])
                ps2 = psp.tile([64, 256, BL], F32, tag="big")
                for c in range(8):
                    nc.tensor.matmul(
                        ps2[:],
                        wpre2[:, c * 64:(c + 1) * 64],
                        e8[:, 16 * c + 512 * n: 16 * c + 512 * n + 512],
                        start=(c == 0), stop=False,
                    )
                nc.tensor.matmul(ps2[:], bias2[:], onerow[:],
                                 start=False, stop=True, skip_group_check=True)
                nc.vector.tensor_copy(preg[0:64, 256 * n:256 * (n + 1), 2:4],
                                      ps2[:])

            # warm up the two g PSUM banks so their has_written bits are set
            # (the in-loop g matmuls use start=False over a DVE pre-write)
            for _ in range(2):
                gw = psp.tile([128, 1, 4], F32, tag="g_ps")
                nc.tensor.matmul(gw[:, 0, :], gse[:], zeros4[:],
                                 start=True, stop=True)

            # ---- the scan ----
            with tc.For_i(0, N, unroll,
                          hint_engines=(mybir.EngineType.PE,)) as iv:
                for k in range(unroll):
                    t = iv + k
                    tcol = t * BL
                    g_ps = psp.tile([128, 1, 4], F32, tag="g_ps")
                    a_ps = psp.tile([64, BL], F32, tag="a_ps", bufs=1)
                    T_psA = psp.tile([128, 16, BL], F32, tag="T_psA", bufs=1)
                    T_psB = psp.tile([128, 16, BL], F32, tag="T_psB", bufs=1)

                    # pre1/pre2 pre-written into the g bank (vector, early)
                    nc.vector.tensor_copy(g_ps[:], preg[:, bass.ds(t, 1), :])

                    # g = [A1|A1]^T m (+pre1), [B1|B1]^T m (+pre2)
                    nc.tensor.matmul(g_ps[:, 0, 0:2], gse[:], m_bf[:],
                                     start=False, stop=True,
                                     skip_group_check=True)
                    nc.tensor.matmul(g_ps[:, 0, 2:4], gso[:], m_bf[:],
                                     start=False, stop=True,
                                     skip_group_check=True)
                    # T chunks: T[p, c, b]: p<64 -> T[h=p, i=c],
                    #                       p>=64 -> T[h=p-64, i=c+32]
                    for c in range(16):
                        nc.tensor.matmul(
                            T_psA[:, c, :],
                            wstatT[:, c * 128:(c + 1) * 128],
                            m_bf[:], start=True, stop=True)
                    for c in range(16, 32):
                        nc.tensor.matmul(
                            T_psB[:, c - 16, :],
                            wstatT[:, c * 128:(c + 1) * 128],
                            m_bf[:], start=True, stop=True)

                    # h = sigmoid(g) (g already includes pre via pre-write)
                    nc.scalar.activation(h1d[:], g_ps[:, 0, 0:2], AF.Sigmoid)
                    nc.scalar.activation(h2t[0:64, :], g_ps[0:64, 0, 2:4],
                                         AF.Sigmoid)

                    # T -> SBUF (bf16); A copy overlaps the B-chunk matmuls
                    nc.vector.tensor_copy(tsb[:, 0:16, :], T_psA[:])
                    nc.vector.tensor_copy(tsb[:, 16:32, :], T_psB[:])

                    # a = WbT@m + bw65@[h2;1] + sum_h h1*T
                    nc.tensor.matmul(a_ps[:], wbT[:], m_bf[:],
                                     start=True, stop=False)
                    nc.tensor.matmul(a_ps[:], bw65[:], h2t[:],
                                     start=False, stop=False,
                                     skip_group_check=True)
                    for b in range(BL):
                        nc.tensor.matmul(a_ps[0:32, b: b + 1],
                                         tsb[0:64, :, b], h1d[0:64, b: b + 1],
                                         start=False, stop=False,
                                         skip_group_check=True,
                                         tile_position=(0, 0))
                        last = b == BL - 1
                        nc.tensor.matmul(a_ps[32:64, b: b + 1],
                                         tsb[64:128, :, b],
                                         h1d[64:128, b: b + 1],
                                         start=False, stop=last,
                                         skip_group_check=True,
                                         tile_position=(64, 32))

                    # m' = sigmoid(a)
                    nc.scalar.activation(m_bf[:], a_ps[:], AF.Sigmoid)
                    nc.vector.tensor_copy(
                        m_hist[0:64, bass.ds(tcol + BL, BL)], m_bf[:])

            # ---- bulk loss ----
            loss = pp.tile([1, NB], F32)
            with tc.tile_pool(name="bulk", bufs=2) as bp:
                for tcn in range(8):
                    sl = slice(512 * tcn, 512 * (tcn + 1))
                    msl = slice(BL + 512 * tcn, BL + 512 * (tcn + 1))
                    se_ps = psp.tile([1, 512], F32, tag="seps", bufs=1)
                    for half in range(2):
                        lg_ps = psp.tile([128, 512], F32, tag="big")
                        exps = bp.tile([128, 512], BF16_DT, tag="exps")
                        nc.tensor.matmul(
                            lg_ps[:],
                            decstat[:, half * 128:(half + 1) * 128],
                            m_hist[:, msl],
                            start=True, stop=True)
                        nc.scalar.activation(exps[:], lg_ps[:], AF.Exp)
                        nc.tensor.matmul(se_ps[:], ones128[:], exps[:],
                                         start=(half == 0), stop=(half == 1))
                    lse_t = bp.tile([1, 512], F32, tag="lse")
                    nc.scalar.activation(lse_t[:], se_ps[:], AF.Ln)
                    paug_t = bp.tile([65, 512], BF16_DT, tag="paug")
                    nc.vector.tensor_tensor(paug_t[:], gaug[:, sl],
                                            m_hist[:, msl],
                                            mybir.AluOpType.mult)
                    pk_ps = psp.tile([1, 512], F32, tag="seps", bufs=1)
                    nc.tensor.matmul(pk_ps[:], ones65[:], paug_t[:],
                                     start=True, stop=True)
                    # loss = lse*inv_ln2 - pick (pick already scaled on host)
                    nc.vector.scalar_tensor_tensor(
                        loss[:, sl], lse_t[:], INV_LN2, pk_ps[:],
                        mybir.AluOpType.mult, mybir.AluOpType.subtract)
            nc.default_dma_engine.dma_start(out_d[:], loss[:])

    nc.compile()
    return nc


def _prep_core_inputs(x0, emb, W_enc_w, W_enc_b, W_dec_w, W_dec_b,
                      b_enc_w, b_enc_b, b_dec_w, b_dec_b, dec_w, dec_b):
    """Host-side gathers/packing -> list of per-core input dicts."""
    f32 = np.float32
    x0 = np.asarray(x0)
    xp = np.concatenate([np.zeros((B, L), x0.dtype), x0], axis=1)  # [B, N+L]
    e = np.asarray(emb, f32)[xp]  # [B, N+L, E]

    # shared weight packs
    Wcat = np.concatenate([np.asarray(W_enc_w, f32), np.asarray(b_enc_w, f32)],
                          axis=1)  # [1088, 128]
    wpre1 = np.zeros((128, 8 * 128), f32)
    wpre2 = np.zeros((128, 8 * 64), f32)
    for c in range(8):
        blk = Wcat[64 + 128 * c: 64 + 128 * (c + 1)]  # [128, 128]
        wpre1[:, c * 128: c * 128 + 64] = blk[:, :64]
        wpre1[:, c * 128 + 64: c * 128 + 128] = blk[:, :64]
        wpre2[:, c * 64:(c + 1) * 64] = blk[:, 64:]
    bias1 = np.concatenate([np.asarray(W_enc_b, f32)] * 2).reshape(1, 128)
    bias2 = np.asarray(b_enc_b, f32).reshape(1, 64)
    gse = np.concatenate([Wcat[:64, :64]] * 2, axis=1)    # [64, 128]
    gso = np.concatenate([Wcat[:64, 64:128]] * 2, axis=1)  # [64, 128]

    W2r = np.asarray(W_dec_w, f32).reshape(H, M, M)  # [h, i, j]
    wstatT = np.zeros((64, 32 * 128), f32)
    for c in range(32):
        wstatT[:, c * 128: c * 128 + 64] = W2r[:, c, :].T
        wstatT[:, c * 128 + 64: c * 128 + 128] = W2r[:, c + 32, :].T
    wbT = np.asarray(W_dec_b, f32).reshape(M, M).T.copy()  # [j, i]
    bw65 = np.concatenate([np.asarray(b_dec_w, f32),
                           np.asarray(b_dec_b, f32).reshape(1, 64)], axis=0)
    decstat = np.concatenate([np.asarray(dec_w, f32),
                              np.asarray(dec_b, f32).reshape(1, 256)], axis=0)
    ones65 = np.ones((65, 1), f32)
    ones128 = np.ones((128, 1), f32)

    shared = dict(
        wpre1=wpre1.astype(BF16), wpre2=wpre2.astype(BF16),
        bias1=bias1.astype(BF16), bias2=bias2.astype(BF16),
        gse=gse.astype(BF16), gso=gso.astype(BF16),
        wstatT=wstatT.astype(FP8), wbT=wbT.astype(BF16),
        bw65=bw65.astype(BF16),
        decstat=decstat.astype(BF16),
        ones65=ones65.astype(BF16), ones128=ones128.astype(BF16),
    )

    in_maps = []
    inv_ln2 = np.float32(1.0 / np.log(2.0))
    dec_wT = np.asarray(dec_w, f32).T.copy()  # [256, 64]
    dec_bv = np.asarray(dec_b, f32)
    for k in range(NCORES):
        rows = slice(BL * k, BL * (k + 1))
        ek = e[rows]  # [BL, N+L, E]
        # e8[l_sub*16+eps, tau*BL+b] = ek[b, tau+l_sub, eps]
        e8 = np.zeros((128, E8COLS), f32)
        for ls in range(8):
            blk = ek[:, ls: ls + TAU, :].transpose(2, 1, 0)  # [E, TAU, BL]
            e8[ls * 16:(ls + 1) * 16] = blk.reshape(E, E8COLS)
        y = np.asarray(x0[rows])  # [BL, N]
        g = dec_wT[y]  # [BL, N, 64]
        gaug = np.zeros((65, NB), f32)
        gaug[:64] = g.transpose(2, 1, 0).reshape(64, NB)
        gaug[64] = dec_bv[y].T.reshape(NB)
        gaug *= inv_ln2
        d = dict(shared)
        d["e8"] = e8.astype(BF16)
        d["gaug"] = gaug.astype(BF16)
        in_maps.append(d)
    return in_maps


def kernel(**inputs):
    key = "nc"
    if key not in _cache:
        _cache[key] = _build_nc()
    nc = _cache[key]
    in_maps = _prep_core_inputs(**inputs)
    res = run_bass_kernel_spmd(nc, in_maps, list(range(NCORES)),
                               trace=bool(os.environ.get("KERNEL_TRACE")))
    _cache["last_result"] = res
    out = np.zeros((N, B), np.float32)
    for k in range(NCORES):
        out[:, BL * k: BL * (k + 1)] = res.results[k]["out"].reshape(N, BL)
    return out.reshape(-1)
